# revision 4
# baseline (speedup 1.0000x reference)
"""Block-dequantized linear (out = x @ (weight * block_scale).T) on 8 TRN2 NeuronCores.

Strategy
--------
Row-parallel over tokens: each of the 8 cores computes a 1024-row slice of the
8192x4096 output against the full (dequantized) weight.

Per core:
  - x slice  [1024, 4096]  kept fully resident in SBUF as K-major tiles (float32r)
  - weight   [4096, 4096]  streamed through SBUF once in 16 column slabs of 256,
    dequantized in place on the vector engine (block scale multiply), then used
    as the stationary matmul operand
  - psum accumulates over the 32 K-tiles; out slice produced transposed
    ([4096, 1024]) and re-transposed on the host at gather time

Matmuls run as float32r (TF32-like single-pass mode, 1 PE cycle/row at free
dim >= 256; measured ~1.4e-4 absmax-relative error at K=4096 vs ~2.4e-3 for
bf16), which is ~4x faster than exact fp32 matmul.

All host-side work is layout only (transpose/tiling for contiguous DMA).
"""

import os

import numpy as np

M, K, N = 8192, 4096, 4096
BLOCK = 128
NCORES = 8
M_CORE = M // NCORES  # 1024
P = 128
KT = K // P  # 32 k-tiles
N_CHUNK = 256  # weight slab width (columns of out)
N_CHUNKS = N // N_CHUNK  # 16
NB = N_CHUNK // P  # n-blocks per slab (2)
MC = 512  # moving-operand width (psum free dim)
MCS = M_CORE // MC  # 2

LAST_EXEC_TIME_NS = None

_PROGRAM = None


def _install_trace_shims():
    """Enable NTFF profiling under axon in images whose `antenv` package lacks
    the axon_hooks module, and stub out the artifact bucket upload."""
    import sys
    import types

    if "antenv.axon_hooks" not in sys.modules:
        from trn_agent_boot.trn_boot import _ntff_profile_via_ctypes

        hook = _ntff_profile_via_ctypes("/opt/axon/libaxon_pjrt.so")
        mod = types.ModuleType("antenv.axon_hooks")
        mod.get_axon_ntff_profile_hook = lambda: hook
        mod.set_axon_ntff_profile_hook = lambda h: None
        sys.modules["antenv.axon_hooks"] = mod
        import antenv

        antenv.axon_hooks = mod

    from concourse import bass_utils

    bass_utils.upload_artifacts = lambda tmpdir: f"local://{tmpdir}"


def _build_program():
    from concourse import bacc
    import concourse.mybir as mybir
    import concourse.tile as tile

    f32 = mybir.dt.float32
    f32r = mybir.dt.float32r

    nc = bacc.Bacc("TRN2", target_bir_lowering=False, debug=False)
    # host layouts (all contiguous):
    #   xt_d[t, p, m] = x[m_core0 + m, t*128 + p]
    #   w_d[c, p, t, j] = weight[c*256 + j, t*128 + p]
    #   s_d[p, g] = scale[g // 32, g % 32]  (replicated over p)
    xt_d = nc.dram_tensor("xt", [KT, P, M_CORE], f32, kind="ExternalInput").ap()
    w_d = nc.dram_tensor("w", [N_CHUNKS, P, KT, N_CHUNK], f32, kind="ExternalInput").ap()
    s_d = nc.dram_tensor("s", [P, (N // BLOCK) * KT], f32, kind="ExternalInput").ap()
    o_d = nc.dram_tensor("o", [N, M_CORE], f32, kind="ExternalOutput").ap()

    with tile.TileContext(nc) as tc:
        with (
            tc.tile_pool(name="xpool", bufs=1) as xpool,
            tc.tile_pool(name="spool", bufs=1) as spool,
            tc.tile_pool(name="wpool", bufs=2) as wpool,
            tc.tile_pool(name="opool", bufs=4) as opool,
            tc.tile_pool(name="pspool", bufs=6, space="PSUM") as pspool,
        ):
            st = spool.tile([P, (N // BLOCK) * KT], f32)
            nc.sync.dma_start(st, s_d)
            xt = xpool.tile([P, KT, M_CORE], f32r)
            for t in range(KT):
                nc.sync.dma_start(xt[:, t, :], xt_d[t].bitcast(f32r))

            for c in range(N_CHUNKS):
                wr = wpool.tile([P, KT, N_CHUNK], f32r, name="wslab")
                nc.sync.dma_start(wr, w_d[c].bitcast(f32r))
                # in-place block dequant: w *= scale[n_block, k_tile]
                for t in range(KT):
                    for nb in range(NB):
                        g = (c * NB + nb) * KT + t
                        nc.vector.tensor_scalar_mul(
                            wr[:, t, nb * P : (nb + 1) * P],
                            wr[:, t, nb * P : (nb + 1) * P],
                            st[:, g : g + 1],
                        )
                for nb in range(NB):
                    for mc in range(MCS):
                        ps = pspool.tile([P, MC], f32, name="ps")
                        for t in range(KT):
                            nc.tensor.matmul(
                                ps,
                                wr[:, t, nb * P : (nb + 1) * P],
                                xt[:, t, mc * MC : (mc + 1) * MC],
                                start=(t == 0),
                                stop=(t == KT - 1),
                            )
                        ost = opool.tile([P, MC], f32, name="ost")
                        nc.scalar.copy(ost, ps)
                        r0 = c * N_CHUNK + nb * P
                        nc.sync.dma_start(
                            o_d[r0 : r0 + P, mc * MC : (mc + 1) * MC], ost
                        )

    nc.compile()
    return nc


def kernel(x, weight, scale):
    global LAST_EXEC_TIME_NS, _PROGRAM
    from concourse import bass_utils

    x = np.ascontiguousarray(np.asarray(x, dtype=np.float32))
    weight = np.ascontiguousarray(np.asarray(weight, dtype=np.float32))
    scale = np.ascontiguousarray(np.asarray(scale, dtype=np.float32))

    # w_d[c, p, t, j] = weight[c*256 + j, t*128 + p]
    w_h = np.ascontiguousarray(
        weight.reshape(N_CHUNKS, N_CHUNK, KT, P).transpose(0, 3, 2, 1)
    )
    # scale broadcast table: column g = nb*KT + kt holds scale[nb, kt] in all partitions
    s_h = np.ascontiguousarray(
        np.broadcast_to(scale.reshape(1, -1), (P, (N // BLOCK) * KT))
    )

    in_maps = []
    for c in range(NCORES):
        xs = x[c * M_CORE : (c + 1) * M_CORE]  # [1024, 4096]
        # xt_d[t, p, m] = xs[m, t*128+p]
        x_h = np.ascontiguousarray(xs.reshape(M_CORE, KT, P).transpose(1, 2, 0))
        in_maps.append({"xt": x_h, "w": w_h, "s": s_h})

    if _PROGRAM is None:
        _PROGRAM = _build_program()
    nc = _PROGRAM

    trace = bool(int(os.environ.get("BASS_KERNEL_TRACE", "0")))
    if trace:
        _install_trace_shims()
    res = bass_utils.run_bass_kernel_spmd(
        nc, in_maps, core_ids=list(range(NCORES)), trace=trace
    )
    LAST_EXEC_TIME_NS = res.exec_time_ns

    out = np.empty((M, N), dtype=np.float32)
    for c in range(NCORES):
        out[c * M_CORE : (c + 1) * M_CORE] = res.results[c]["o"].T
    return out


# revision 5
# speedup vs baseline: 1.0495x; 1.0495x over previous
"""Block-dequantized linear (out = x @ (weight * block_scale).T) on 8 TRN2 NeuronCores.

Strategy
--------
Row-parallel over tokens: each of the 8 cores computes a 1024-row slice of the
8192x4096 output against the full (dequantized) weight.

Per core:
  - x slice  [1024, 4096]  kept fully resident in SBUF as K-major fp16 tiles
  - weight   [4096, 4096]  streamed through SBUF once in 8 fp16 column slabs of
    512, dequantized in place on the vector engine (single stride-0-broadcast
    tensor_tensor multiply per slab), then used as the stationary matmul operand
  - psum accumulates in fp32 over the 32 K-tiles; the out slice is produced
    transposed ([4096, 1024]) and re-transposed on the host at gather time

fp16 operands: measured 2.9e-4 absmax-relative error at K=4096 (products are
exact in the fp32 PSUM accumulate; only input rounding contributes), while the
PE runs at full 1-cycle/row rate with fast weight loads — unlike fp32/float32r,
whose weight loads halve throughput (measured: f32r 552us vs fp16 ~270us/core).

All host-side work is layout/dtype only (transpose/tiling for contiguous DMA).
"""

import os

import numpy as np

M, K, N = 8192, 4096, 4096
BLOCK = 128
NCORES = 8
M_CORE = M // NCORES  # 1024
P = 128
KT = K // P  # 32 k-tiles
N_CHUNK = 512  # weight slab width (columns of out)
N_CHUNKS = N // N_CHUNK  # 8
NB = N_CHUNK // P  # n-blocks per slab (4)
MC = 512  # moving-operand width (psum free dim)
MCS = M_CORE // MC  # 2
NBG = N // BLOCK  # 32 global n-blocks

LAST_EXEC_TIME_NS = None

_PROGRAM = None


def _install_trace_shims():
    """Enable NTFF profiling under axon in images whose `antenv` package lacks
    the axon_hooks module, and stub out the artifact bucket upload."""
    import sys
    import types

    if "antenv.axon_hooks" not in sys.modules:
        from trn_agent_boot.trn_boot import _ntff_profile_via_ctypes

        hook = _ntff_profile_via_ctypes("/opt/axon/libaxon_pjrt.so")
        mod = types.ModuleType("antenv.axon_hooks")
        mod.get_axon_ntff_profile_hook = lambda: hook
        mod.set_axon_ntff_profile_hook = lambda h: None
        sys.modules["antenv.axon_hooks"] = mod
        import antenv

        antenv.axon_hooks = mod

    from concourse import bass_utils

    bass_utils.upload_artifacts = lambda tmpdir: f"local://{tmpdir}"


def _build_program():
    from concourse import bacc
    import concourse.mybir as mybir
    import concourse.tile as tile

    f16 = mybir.dt.float16
    f32 = mybir.dt.float32

    nc = bacc.Bacc("TRN2", target_bir_lowering=False, debug=False)
    # host layouts (all contiguous):
    #   xt_d[t, p, m] = x[m_core0 + m, t*128 + p]        (fp16)
    #   w_d[c, p, t, j] = weight[c*512 + j, t*128 + p]   (fp16)
    #   s_d[p, g] = scale[g // KT, g % KT]               (fp32, replicated over p)
    xt_d = nc.dram_tensor("xt", [KT, P, M_CORE], f16, kind="ExternalInput").ap()
    w_d = nc.dram_tensor("w", [N_CHUNKS, P, KT, N_CHUNK], f16, kind="ExternalInput").ap()
    s_d = nc.dram_tensor("s", [P, NBG * KT], f32, kind="ExternalInput").ap()
    o_d = nc.dram_tensor("o", [N, M_CORE], f32, kind="ExternalOutput").ap()

    with tile.TileContext(nc) as tc:
        with (
            tc.tile_pool(name="xpool", bufs=1) as xpool,
            tc.tile_pool(name="spool", bufs=1) as spool,
            tc.tile_pool(name="wpool", bufs=2) as wpool,
            tc.tile_pool(name="opool", bufs=4) as opool,
            tc.tile_pool(name="pspool", bufs=6, space="PSUM") as pspool,
        ):
            st = spool.tile([P, NBG * KT], f32)
            nc.sync.dma_start(st, s_d)
            xt = xpool.tile([P, KT, M_CORE], f16)
            for t in range(KT):
                nc.sync.dma_start(xt[:, t, :], xt_d[t])

            for c in range(N_CHUNKS):
                wr = wpool.tile([P, KT, N_CHUNK], f16, name="wslab")
                nc.sync.dma_start(wr, w_d[c])
                # in-place block dequant: w[p, t, nb*128 + j] *= scale[c*NB + nb, t]
                w_v = wr.rearrange("p t (nb j) -> p t nb j", j=P)
                s_v = (
                    st[:, c * NB * KT : (c * NB + NB) * KT]
                    .rearrange("p (nb t) -> p t nb", t=KT)
                    .broadcast_to([P, KT, NB, P])
                )
                nc.vector.tensor_mul(w_v, w_v, s_v)
                for nb in range(NB):
                    for mc in range(MCS):
                        ps = pspool.tile([P, MC], f32, name="ps")
                        for t in range(KT):
                            nc.tensor.matmul(
                                ps,
                                wr[:, t, nb * P : (nb + 1) * P],
                                xt[:, t, mc * MC : (mc + 1) * MC],
                                start=(t == 0),
                                stop=(t == KT - 1),
                            )
                        ost = opool.tile([P, MC], f32, name="ost")
                        nc.scalar.copy(ost, ps)
                        r0 = c * N_CHUNK + nb * P
                        nc.sync.dma_start(
                            o_d[r0 : r0 + P, mc * MC : (mc + 1) * MC], ost
                        )

    nc.compile()
    return nc


def kernel(x, weight, scale):
    global LAST_EXEC_TIME_NS, _PROGRAM
    from concourse import bass_utils

    x = np.asarray(x, dtype=np.float32)
    weight = np.asarray(weight, dtype=np.float32)
    scale = np.ascontiguousarray(np.asarray(scale, dtype=np.float32))

    # w_d[c, p, t, j] = weight[c*512 + j, t*128 + p]
    w_h = np.ascontiguousarray(
        weight.astype(np.float16).reshape(N_CHUNKS, N_CHUNK, KT, P).transpose(0, 3, 2, 1)
    )
    # scale table: column g = nb*KT + t holds scale[nb, t] in all partitions
    s_h = np.ascontiguousarray(
        np.broadcast_to(scale.reshape(1, NBG * KT), (P, NBG * KT))
    )

    x16 = x.astype(np.float16)
    in_maps = []
    for c in range(NCORES):
        xs = x16[c * M_CORE : (c + 1) * M_CORE]  # [1024, 4096]
        # xt_d[t, p, m] = xs[m, t*128+p]
        x_h = np.ascontiguousarray(xs.reshape(M_CORE, KT, P).transpose(1, 2, 0))
        in_maps.append({"xt": x_h, "w": w_h, "s": s_h})

    if _PROGRAM is None:
        _PROGRAM = _build_program()
    nc = _PROGRAM

    trace = bool(int(os.environ.get("BASS_KERNEL_TRACE", "0")))
    if trace:
        _install_trace_shims()
    res = bass_utils.run_bass_kernel_spmd(
        nc, in_maps, core_ids=list(range(NCORES)), trace=trace
    )
    LAST_EXEC_TIME_NS = res.exec_time_ns

    out = np.empty((M, N), dtype=np.float32)
    for c in range(NCORES):
        out[c * M_CORE : (c + 1) * M_CORE] = res.results[c]["o"].T
    return out
